# revision 26
# baseline (speedup 1.0000x reference)
"""Multi-head self-attention forward on 8 Trainium2 NeuronCores.

Problem: x[4, 2048, 1024] fp32, weights wq/wk/wv/wo [1024, 1024].
  Q,K,V = x @ w{q,k,v}.T (16 heads x 64); causal softmax(QK^T/8)V; out @ wo.T.

Sharding (single SPMD program, per-core data differs only):
  core c: batch b = c//2, head-half hh = c%2 (heads hh*8..hh*8+8),
  wo-half hh (output dims hh*512..). Per pair (2b, 2b+1):
    - each core: Q/K/V projections for its 8 heads (full 2048 tokens),
      causal flash attention for those heads, producing O^T [512, 2048]
    - pairwise AllGather of O^T chunks -> O_full^T [1024, 2048]
    - each core: out-proj against its 512 output dims -> z [2048, 512]
  Host unshard: out[b][:, hh*512:] = core(2b+hh) output.

Schedule: the exp (scalar/ACT engine) is the attention-phase floor while the
PE owns the projections, so QT/KT projection chains for the next head pair's
chunk and out-projection partials for completed gathers are interleaved as
fillers into the attention emission; the PV of each score group is emitted
one group late so the exp latency hides under the next group's scores.
Chunk 3 is exchanged in per-qb quarters so the final collective only covers
512 query columns.

Perf notes baked in:
  - all matmuls full 128-row: per-head K^T/Q^T are zero-padded to 128
    contraction rows via SBUF->SBUF DMA staging (a 64-row stationary makes
    the following LDWEIGHTS wait out the drain, ~300ns per transition).
  - softmax denominators ride along in the PV matmul via Vaug = [V_h | ones],
    divide = copy + reciprocal_approx_fast (base-partition-0 only) + mult.
  - causal mask seeded into PSUM with PE matmuls (identity @ mask), emitted
    before the score matmuls of a group so identity stays stationary.
  - out-proj partials accumulate in SBUF (bf16 A/B rotation) per gather.
"""

import sys

sys.path.insert(0, "/opt/trn_rl_repo")

import ml_dtypes
import numpy as np
import concourse.bass as bass
import concourse.mybir as mybir
import concourse.tile as tile
from concourse import bacc
from concourse.bass_utils import run_bass_kernel_spmd
from concourse.masks import make_identity

F32 = mybir.dt.float32
BF16 = mybir.dt.bfloat16
AF = mybir.ActivationFunctionType
OP = mybir.AluOpType

N_CORES = 8
S = 2048          # sequence length
D = 1024          # model dim
HL = 8            # heads per core
DK = 64           # head dim
DL = HL * DK      # local head dims = 512
NEG = -1e30
GK = 2            # k-chunks per psum_s group

_NC_CACHE = {}


def build():
    nc = bacc.Bacc("TRN2", target_bir_lowering=False, debug=False, num_devices=N_CORES)

    xb = nc.dram_tensor("xb", [S, D], BF16, kind="ExternalInput")
    wq = nc.dram_tensor("wq", [DL, D], BF16, kind="ExternalInput")
    wk = nc.dram_tensor("wk", [DL, D], BF16, kind="ExternalInput")
    wv = nc.dram_tensor("wv", [DL, D], BF16, kind="ExternalInput")
    wo = nc.dram_tensor("wo", [DL, D], BF16, kind="ExternalInput")  # out-dim half
    mtri = nc.dram_tensor("mtri", [128, 128], F32, kind="ExternalInput")
    z = nc.dram_tensor("z", [S, DL], F32, kind="ExternalOutput")

    with tile.TileContext(nc) as tc:
        with (
            tc.tile_pool(name="cst", bufs=1) as cst,
            tc.tile_pool(name="per", bufs=1) as per,
            tc.tile_pool(name="dram", bufs=1, space="DRAM") as dram,
            tc.tile_pool(name="pps", bufs=2, space="PSUM") as pps,
        ):
            # ---------- constants ----------
            ident = cst.tile([128, 128], F32)
            make_identity(nc, ident[:])
            identb = cst.tile([128, 128], BF16)
            nc.vector.tensor_copy(identb[:], ident[:])
            mt = cst.tile([128, 128], F32)
            nc.sync.dma_start(mt[:], mtri[:])
            mtb = cst.tile([128, 128], BF16)
            nc.vector.tensor_copy(mtb[:], mt[:])

            # persistent across attn -> out-proj
            OT = per.tile([128, 4, S], BF16)     # [p(dl in chunk), chunk, q]
            woT = per.tile([128, 8, DL], BF16)   # [p(din in chunk), chunk, dout]
            zA = per.tile([128, 16, DL], BF16)   # out-proj partial accum (A)
            zB = per.tile([128, 16, DL], BF16)   # out-proj partial accum (B)

            cins = []
            gouts = []
            for j in range(3):
                cin_t = dram.tile([128, S], BF16, tag=f"cin{j}", name=f"cin{j}")
                gout_t = dram.tile([256, S], BF16, tag=f"gout{j}", name=f"gout{j}")
                cins.append(cin_t)
                gouts.append(gout_t)
            # chunk 3 is exchanged in per-qb quarters so the last collective
            # only covers 512 query columns
            cin3q = [dram.tile([128, 512], BF16, tag=f"cin3q{j}", name=f"cin3q{j}")
                     for j in range(4)]
            gout3q = [dram.tile([256, 512], BF16, tag=f"gout3q{j}", name=f"gout3q{j}")
                      for j in range(4)]

            with (
                tc.tile_pool(name="qkv", bufs=1) as qkv,
                tc.tile_pool(name="wtp", bufs=1) as wtp,
                tc.tile_pool(name="xtp", bufs=1) as xtp,
            ):
                QT = qkv.tile([128, 4, S], BF16)      # [p, dlocal chunk, q]
                KT = qkv.tile([128, 4, S], BF16)
                # Vaug: [p(tok in kc), h, kc, 0:64]=V, [.., 64:128]=ones
                VA = qkv.tile([128, 8, 16, 128], BF16)
                wTq = wtp.tile([128, 8, DL], BF16)
                wTk = wtp.tile([128, 8, DL], BF16)
                xT = xtp.tile([128, 8, S], BF16)  # [p(din in chunk), chunk, tok]

                # ones half of Vaug never changes: fill once
                nc.gpsimd.memset(VA[:, :, :, DK:128], 1.0)

                # ---- projection chain emitters (used for chunk 0 directly,
                # chunks 1..3 as attention fillers) ----
                def proj_chain(wT, dst, c, tb, scale):
                    def emit():
                        pp = pps.tile([128, 512], F32, tag="pp")
                        for i in range(8):
                            nc.tensor.matmul(
                                pp[:],
                                wT[:, i, c * 128:(c + 1) * 128],
                                xT[:, i, tb * 512:(tb + 1) * 512],
                                start=(i == 0), stop=(i == 7))
                        if scale is None:
                            nc.vector.tensor_copy(
                                dst[:, c, tb * 512:(tb + 1) * 512], pp[:])
                        else:
                            nc.vector.tensor_scalar_mul(
                                dst[:, c, tb * 512:(tb + 1) * 512], pp[:], scale)
                    return emit

                def proj_chunk_fillers(c):
                    out = []
                    for wT, dst, scale in ((wTk, KT, None), (wTq, QT, 0.125)):
                        for tb in range(4):
                            out.append(proj_chain(wT, dst, c, tb, scale))
                    return out

                # ---- phase 1: transposes (PE), V proj, chunk-0 Q/K proj ----
                with (
                    tc.tile_pool(name="wvp", bufs=1) as wvp,
                    tc.tile_pool(name="natp", bufs=2) as natp,
                    tc.tile_pool(name="wps", bufs=2, space="PSUM") as wps,
                ):
                    wTv = wvp.tile([128, 8, DL], BF16)

                    def transpose_in(src_dram, dst):
                        # src [512,1024] bf16 -> dst[:, i, :] transposed;
                        # one batched strided DMA per source
                        nat = natp.tile([128, 4, D], BF16, tag="nat", name="nat")
                        nc.sync.dma_start(
                            nat[:], src_dram[:].rearrange("(r p) d -> p r d", p=128))
                        for i in range(8):
                            pw = wps.tile([128, 512], BF16, tag="wtp")
                            for r in range(4):
                                nc.tensor.transpose(
                                    pw[:, r * 128:(r + 1) * 128],
                                    nat[:, r, i * 128:(i + 1) * 128],
                                    identb[:])
                            nc.vector.tensor_copy(dst[:, i, :], pw[:])

                    transpose_in(wv, wTv)
                    for tb in range(4):
                        transpose_in(xb[tb * 512:(tb + 1) * 512, :],
                                     xT[:, :, tb * 512:(tb + 1) * 512])

                    # V proj: VA[:, h, r, 0:64] for all h in one strided DVE
                    # copy per token chunk r
                    for r in range(16):
                        pp = pps.tile([128, 512], F32, tag="pp")
                        for i in range(8):
                            nc.tensor.matmul(
                                pp[:],
                                xT[:, i, r * 128:(r + 1) * 128],
                                wTv[:, i, :],
                                start=(i == 0), stop=(i == 7))
                        nc.vector.tensor_copy(
                            VA[:, :, r, 0:DK],
                            pp[:].rearrange("p (h d) -> p h d", h=HL))

                    transpose_in(wq, wTq)
                    transpose_in(wk, wTk)
                    transpose_in(wo, woT)

                    for f in proj_chunk_fillers(0):
                        f()

                # ---- attention + interleaved fillers ----
                with (
                    tc.tile_pool(name="qkpad", bufs=2) as qkpad,
                    tc.tile_pool(name="ptp", bufs=3) as ptp,
                    tc.tile_pool(name="dvp", bufs=1) as dvp,
                    tc.tile_pool(name="otfp", bufs=1) as otfp,
                    tc.tile_pool(name="zsb", bufs=2) as zsb,
                    tc.tile_pool(name="aps", bufs=2, space="PSUM") as aps,
                    tc.tile_pool(name="apo", bufs=2, space="PSUM") as apo,
                ):
                    # out-proj partial pass for gather g (g = chunk index).
                    # zacc rotation: g0 -> zA, g1 -> zB, g2 -> zA, g3 reads
                    # zA, adds last psum, writes z. For g=3 the chunk is
                    # exchanged in per-qb quarters (qr = 0..3).
                    def gather_fillers(g, qr=None):
                        if qr is None:
                            src, cols, qts = gouts[g], S, range(16)
                        else:
                            src, cols, qts = gout3q[qr], 512, range(4 * qr, 4 * qr + 4)
                        ofA = otfp.tile([128, S], BF16, tag="ofA")
                        ofB = otfp.tile([128, S], BF16, tag="ofB")

                        def emit_dma():
                            nc.sync.dma_start(ofA[:, 0:cols], src[0:128, :])
                            nc.sync.dma_start(ofB[:, 0:cols], src[128:256, :])
                        out = [emit_dma]

                        def qt_step(qt, lt):
                            def emit():
                                pz = pps.tile([128, DL], F32, tag="pp")
                                nc.tensor.matmul(
                                    pz[:], ofA[:, lt * 128:(lt + 1) * 128],
                                    woT[:, g, :], start=True, stop=False)
                                nc.tensor.matmul(
                                    pz[:], ofB[:, lt * 128:(lt + 1) * 128],
                                    woT[:, g + 4, :], start=False, stop=True)
                                if g == 0:
                                    nc.vector.tensor_copy(zA[:, qt, :], pz[:])
                                elif g == 1:
                                    nc.vector.tensor_tensor(
                                        zB[:, qt, :], pz[:], zA[:, qt, :], OP.add)
                                elif g == 2:
                                    nc.vector.tensor_tensor(
                                        zA[:, qt, :], pz[:], zB[:, qt, :], OP.add)
                                else:
                                    zt = zsb.tile([128, DL], F32, tag="zt")
                                    nc.vector.tensor_tensor(
                                        zt[:], pz[:], zA[:, qt, :], OP.add)
                                    nc.sync.dma_start(
                                        z[qt * 128:(qt + 1) * 128, :], zt[:])
                            return emit
                        out.extend(qt_step(qt, lt) for lt, qt in enumerate(qts))
                        return out

                    def emit_pv(h, prev, po_t, nkc):
                        kcs, offs, pt_t, q0 = prev
                        for off, kc in zip(offs, kcs):
                            ws = max(0, kc * 128 - q0)
                            W = 512 - ws
                            nc.tensor.matmul(
                                po_t[:, ws:512],
                                VA[:, h, kc, :],
                                pt_t[:, off: off + W],
                                start=(kc == 0), stop=(kc == nkc - 1))

                    def stage_kq(h, memset_pad):
                        # zero-pad K^T/Q^T of head h to 128 contraction rows
                        # (slots rotate; rows 64:128 only need zeroing the
                        # first time each slot is used)
                        ch = h // 2
                        po = (h % 2) * 64
                        kth = qkpad.tile([128, S], BF16, tag="kth")
                        qth = qkpad.tile([128, S], BF16, tag="qth")
                        if memset_pad:
                            nc.gpsimd.memset(kth[64:128, :], 0.0)
                            nc.gpsimd.memset(qth[64:128, :], 0.0)
                        nc.sync.dma_start(kth[0:64, :], KT[po:po + 64, ch, :])
                        nc.sync.dma_start(qth[0:64, :], QT[po:po + 64, ch, :])
                        return kth, qth

                    def head_qb(h, qb, kth, qth, fillers):
                        ch = h // 2
                        po = (h % 2) * 64
                        q0 = qb * 512
                        nkc = 4 * (qb + 1)
                        po_t = apo.tile([128, 512], F32, tag="po")
                        prev = None
                        for g0 in range(0, nkc, GK):
                            kcs = list(range(g0, min(g0 + GK, nkc)))
                            ps = aps.tile([128, GK * 512], F32, tag="ps")
                            pt = ptp.tile([128, GK * 512], BF16, tag="pt")
                            offs = [i * 512 for i in range(len(kcs))]
                            o = offs[-1] + 512 - max(0, kcs[-1] * 128 - q0)
                            for off, kc in zip(offs, kcs):
                                ws = max(0, kc * 128 - q0)
                                nc.tensor.matmul(
                                    ps[:, off: off + 512 - ws],
                                    kth[:, kc * 128:(kc + 1) * 128],
                                    qth[:, q0 + ws: q0 + 512],
                                    start=True, stop=True)
                            nc.scalar.activation(
                                pt[:, 0:o], ps[:, 0:o], AF.Exp)
                            # causal mask applied post-exp on the DVE: the
                            # first 128 written columns of a diagonal chunk
                            # are an (inclusive) upper triangle
                            for off, kc in zip(offs, kcs):
                                if kc * 128 >= q0:
                                    nc.vector.tensor_tensor(
                                        pt[:, off: off + 128],
                                        pt[:, off: off + 128],
                                        mtb[:], OP.mult)
                            # PV of the previous group -- its exp finished
                            # while this group's scores streamed
                            if prev is not None:
                                emit_pv(h, prev, po_t, nkc)
                            if fillers:
                                fillers.pop(0)()
                            prev = (kcs, offs, pt, q0)
                        emit_pv(h, prev, po_t, nkc)
                        # divide by softmax sum (rows 64:128 of po).
                        # reciprocal_approx_fast requires base partition 0
                        # for its input, so stage the sums via a copy.
                        den = dvp.tile([64, 512], F32, tag="den")
                        nc.vector.tensor_copy(den[:], po_t[64:128, :])
                        rec = dvp.tile([64, 512], F32, tag="rec")
                        nc.vector.reciprocal_approx_fast(rec[:], den[:])
                        nc.vector.tensor_tensor(
                            OT[po:po + 64, ch, q0:q0 + 512],
                            po_t[0:64, :], rec[:], OP.mult)

                    def exchange(ins_t, outs_t):
                        nc.gpsimd.collective_compute(
                            "AllGather", OP.bypass,
                            replica_groups=[[0, 1], [2, 3], [4, 5], [6, 7]],
                            ins=[ins_t[:]], outs=[outs_t[:]])

                    # chunks 0-2: heads sequential; proj chains for the next
                    # chunk fill the even head, the previous chunk's gather
                    # pass fills the odd head (its collective has then had a
                    # full head's time to land)
                    for h in range(6):
                        ch = h // 2
                        fillers = []
                        if h % 2 == 0 and ch < 3:
                            fillers.extend(proj_chunk_fillers(ch + 1))
                        if h % 2 == 1 and ch >= 1:
                            fillers.extend(gather_fillers(ch - 1))
                        kth, qth = stage_kq(h, memset_pad=(h < 2))
                        for qb in range(4):
                            head_qb(h, qb, kth, qth, fillers)
                        for f in fillers:
                            f()
                        if h % 2 == 1:
                            nc.sync.dma_start(cins[ch][:], OT[:, ch, :])
                            exchange(cins[ch], gouts[ch])

                    # chunk 3: heads 6 and 7 interleaved per qb, so each
                    # quarter of OT[:, 3, :] completes (and its exchange
                    # fires) as early as possible -- the last collective then
                    # lands right at the end of attention
                    fillers = []
                    k6, q6 = stage_kq(6, memset_pad=False)
                    k7, q7 = stage_kq(7, memset_pad=False)
                    for qb in range(4):
                        if qb == 1:
                            fillers.extend(gather_fillers(2))
                        if qb >= 2:
                            fillers.extend(gather_fillers(3, qr=qb - 2))
                        head_qb(6, qb, k6, q6, fillers)
                        head_qb(7, qb, k7, q7, fillers)
                        nc.sync.dma_start(cin3q[qb][:], OT[:, 3, qb * 512:(qb + 1) * 512])
                        exchange(cin3q[qb], gout3q[qb])
                    fillers.extend(gather_fillers(3, qr=2))
                    for f in fillers:
                        f()

                    # ---- tail: last quarter of chunk 3 ----
                    for f in gather_fillers(3, qr=3):
                        f()

    nc.compile()
    return nc


def _get_nc():
    if "nc" not in _NC_CACHE:
        _NC_CACHE["nc"] = build()
    return _NC_CACHE["nc"]


def kernel(x, wq, wk, wv, wo, _trace=False):
    bf = ml_dtypes.bfloat16
    x = np.ascontiguousarray(np.asarray(x, dtype=np.float32)).astype(bf)
    wq = np.ascontiguousarray(np.asarray(wq, dtype=np.float32)).astype(bf)
    wk = np.ascontiguousarray(np.asarray(wk, dtype=np.float32)).astype(bf)
    wv = np.ascontiguousarray(np.asarray(wv, dtype=np.float32)).astype(bf)
    wo = np.ascontiguousarray(np.asarray(wo, dtype=np.float32)).astype(bf)
    b, s, d = x.shape
    assert (b, s, d) == (4, S, D)

    # inclusive upper triangle: key k may attend query i iff k <= i
    mtri = (np.arange(128)[:, None] <= np.arange(128)[None, :]).astype(np.float32)

    in_maps = []
    for c in range(N_CORES):
        bi, hh = c // 2, c % 2
        in_maps.append({
            "xb": x[bi],
            "wq": wq[hh * DL:(hh + 1) * DL, :],
            "wk": wk[hh * DL:(hh + 1) * DL, :],
            "wv": wv[hh * DL:(hh + 1) * DL, :],
            "wo": wo[hh * DL:(hh + 1) * DL, :],
            "mtri": mtri,
        })

    nc = _get_nc()
    res = run_bass_kernel_spmd(nc, in_maps, core_ids=list(range(N_CORES)),
                               trace=_trace)

    out = np.empty((4, S, D), dtype=np.float32)
    for c in range(N_CORES):
        bi, hh = c // 2, c % 2
        out[bi][:, hh * DL:(hh + 1) * DL] = res.results[c]["z"]
    if _trace:
        kernel.last_exec_time_ns = res.exec_time_ns
    return out
